# revision 15
# baseline (speedup 1.0000x reference)
"""Chamfer distance loss kernel for Trainium2 (8 NeuronCores, SPMD).

Problem: x1 [16, 4096, 3], x2 [16, 4096, 3] ->
    chamfer[b] = mean_i min_j ||x1[b,i]-x2[b,j]||^2 + mean_j min_i ||...||^2

Strategy (v3):
  - Data-parallel over batch: 2 batches per core.
  - Distance embedding: u_i = [x1_i, |x1_i|^2, 1], v_j = [-2*x2_j, 1, |x2_j|^2]
    => u_i . v_j = ||x1_i - x2_j||^2; one [K, 128] x [K, FD] matmul produces a
    128 x FD distance block in PSUM.
  - fp32 matmuls stream at 1/4 rate on the PE, so inputs are split into 3
    bfloat16 components and K carries the full 3x3 outer product (K = 45):
    exactly (uh+um+ul).(vh+vm+vl) ~ fp32-accurate at bf16 streaming rate.
  - Single generation per batch serves BOTH reductions: ScalarE copies each
    PSUM tile to fp16 SBUF; VectorE runs 2x-rate fp16 tensor-tensor mins:
      row-min:  binary folds over j then one 1x reduce per i-chunk
      col-min:  elementwise running fold across i-chunks into runmin[128,4096]
    Col-min finishes with PE 128x128 transposes + one batched reduce
    (partition residue -> free axis).
  - Host folds the small [128, 32] slot tensors into the final [16] means.
"""

import sys

for _p in ("/opt/trn_rl_repo",):
    if _p not in sys.path:
        sys.path.insert(0, _p)

import ml_dtypes
import numpy as np

B, N, M = 16, 4096, 4096
NCORES = 8
BPC = B // NCORES  # batches per core
K = 5  # embedding dim; K3 = 3 bf16 splits x 3 = 45 matmul contraction
K3 = 9 * K
P = 128  # partitions
SPAN = 2048  # distance elements per PSUM tile (4 banks)
NSPAN = M // SPAN  # 2 spans per chunk
MMBLK = 512  # matmul free dim (1 PSUM bank)
NCHUNK = N // P  # 32 chunks of the i-side
NTP = SPAN // P  # 16 transpose blocks per runmin tile

_built = {}


def _build_nc(repeat=1):
    import concourse.bacc as bacc
    import concourse.mybir as mybir
    import concourse.tile as tile

    f32 = mybir.dt.float32
    bf16 = mybir.dt.bfloat16
    fp16 = mybir.dt.float16

    nc = bacc.Bacc(
        "TRN2", target_bir_lowering=False, debug=False, num_devices=NCORES
    )
    ut3_ext = nc.dram_tensor("ut3", [BPC, K3, N], bf16, kind="ExternalInput").ap()
    vr3_ext = nc.dram_tensor("vr3", [BPC, K3, M], bf16, kind="ExternalInput").ap()
    ra_ext = nc.dram_tensor(
        "rowacc", [BPC, P, NCHUNK], f32, kind="ExternalOutput"
    ).ap()
    ca_ext = nc.dram_tensor(
        "colacc", [BPC, P, M // P], f32, kind="ExternalOutput"
    ).ap()

    with tile.TileContext(nc) as tc:
        with (
            tc.tile_pool(name="const", bufs=1) as cpool,
            tc.tile_pool(name="uv", bufs=1) as uvpool,
            tc.tile_pool(name="acc", bufs=1) as apool,
            tc.tile_pool(name="work", bufs=4) as wpool,
            tc.tile_pool(name="psum", bufs=2, space="PSUM") as ppool,
        ):
            from concourse import masks

            ident = cpool.tile([P, P], fp16, tag="ident", name="ident")
            masks.make_identity(nc, ident[:])

            def body():
                _body(
                    nc, mybir, uvpool, apool, wpool, ppool, ident,
                    (ut3_ext, vr3_ext), (ra_ext, ca_ext),
                )

            if repeat == 1:
                body()
            else:
                with tc.For_i(0, repeat, 1):
                    body()
    nc.compile()
    return nc


def _body(nc, mybir, uvpool, apool, wpool, ppool, ident, ins, outs):
    f32 = mybir.dt.float32
    bf16 = mybir.dt.bfloat16
    fp16 = mybir.dt.float16
    mn = mybir.AluOpType.min
    X = mybir.AxisListType.X
    ut3_ext, vr3_ext = ins
    ra_ext, ca_ext = outs
    for b in range(BPC):
        Ut = uvpool.tile([K3, N], bf16, tag=f"ut{b}", name=f"ut{b}")
        Vr = uvpool.tile([K3, M], bf16, tag=f"vr{b}", name=f"vr{b}")
        nc.sync.dma_start(Ut[:], ut3_ext[b])
        nc.sync.dma_start(Vr[:], vr3_ext[b])
        rowacc = apool.tile([P, NCHUNK], f32, tag=f"ra{b}", name=f"ra{b}")
        colacc = apool.tile([P, M // P], f32, tag=f"ca{b}", name=f"ca{b}")
        runmin = apool.tile([P, M], fp16, tag=f"rm{b}", name=f"rm{b}")
        for c in range(NCHUNK):
            cp = wpool.tile([P, M], fp16, tag="cp", name="cp")
            for sp in range(NSPAN):
                dist = ppool.tile([P, SPAN], f32, tag="dist", name="dist")
                for h in range(SPAN // MMBLK):
                    j0 = sp * SPAN + h * MMBLK
                    nc.tensor.matmul(
                        dist[:, h * MMBLK : (h + 1) * MMBLK],
                        Ut[:, c * P : (c + 1) * P],
                        Vr[:, j0 : j0 + MMBLK],
                        start=True,
                        stop=True,
                    )
                nc.scalar.copy(cp[:, sp * SPAN : (sp + 1) * SPAN], dist[:])
            # col-min: running elementwise fold across i-chunks
            if c == 0:
                nc.vector.tensor_copy(runmin[:], cp[:])
            else:
                nc.vector.tensor_tensor(
                    out=runmin[:], in0=cp[:], in1=runmin[:], op=mn
                )
            # row-min: binary folds over j, then one small reduce
            rowf = wpool.tile([P, M // 2], fp16, tag="rowf", name="rowf")
            nc.vector.tensor_tensor(
                out=rowf[:], in0=cp[:, : M // 2], in1=cp[:, M // 2 :], op=mn
            )
            rowf2 = wpool.tile([P, M // 4], fp16, tag="rowf2", name="rowf2")
            nc.vector.tensor_tensor(
                out=rowf2[:], in0=rowf[:, : M // 4], in1=rowf[:, M // 4 :], op=mn
            )
            rowf3 = wpool.tile([P, M // 8], fp16, tag="rowf3", name="rowf3")
            nc.vector.tensor_tensor(
                out=rowf3[:], in0=rowf2[:, : M // 8], in1=rowf2[:, M // 8 :], op=mn
            )
            nc.vector.tensor_reduce(
                out=rowacc[:, c : c + 1], in_=rowf3[:], axis=X, op=mn
            )
        # col-min finalize: partition residue -> free axis via PE transpose
        for sp in range(NSPAN):
            tp = ppool.tile([P, SPAN], fp16, tag="dist", name="tp")
            for t in range(NTP):
                nc.tensor.transpose(
                    tp[:, t * P : (t + 1) * P],
                    runmin[:, sp * SPAN + t * P : sp * SPAN + (t + 1) * P],
                    ident[:],
                )
            nc.vector.tensor_reduce(
                out=colacc[:, sp * NTP : (sp + 1) * NTP],
                in_=tp[:].rearrange("p (t x) -> p t x", x=P),
                axis=X,
                op=mn,
            )
        nc.sync.dma_start(ra_ext[b], rowacc[:])
        nc.sync.dma_start(ca_ext[b], colacc[:])


def _split3(a):
    """Split fp32 array into 3 bf16 components summing to ~a (fp32 accurate)."""
    a = np.asarray(a, np.float32)
    h = a.astype(ml_dtypes.bfloat16)
    r = a - h.astype(np.float32)
    m = r.astype(ml_dtypes.bfloat16)
    l = (r - m.astype(np.float32)).astype(ml_dtypes.bfloat16)
    return h, m, l


def _prep_in_maps(x1: np.ndarray, x2: np.ndarray):
    x1 = np.asarray(x1, dtype=np.float32)
    x2 = np.asarray(x2, dtype=np.float32)
    # center the clouds (chamfer is translation invariant; shrinks magnitudes
    # so the bf16-split dot keeps more effective precision)
    ctr = 0.5 * (x1.mean(axis=(1,), keepdims=True) + x2.mean(axis=(1,), keepdims=True))
    x1c = x1 - ctr
    x2c = x2 - ctr
    n1 = (x1c.astype(np.float64) ** 2).sum(-1).astype(np.float32)  # [B, N]
    n2 = (x2c.astype(np.float64) ** 2).sum(-1).astype(np.float32)  # [B, M]
    u_all = np.concatenate(
        [x1c.transpose(0, 2, 1), n1[:, None, :], np.ones((B, 1, N), np.float32)],
        axis=1,
    )  # [B, 5, N]
    v_all = np.concatenate(
        [
            -2.0 * x2c.transpose(0, 2, 1),
            np.ones((B, 1, M), np.float32),
            n2[:, None, :],
        ],
        axis=1,
    )  # [B, 5, M]
    uh, um, ul = _split3(u_all)
    vh, vm, vl = _split3(v_all)
    usplit = np.concatenate([uh, um, ul], axis=1)  # [B, 15, N]
    ut3 = np.tile(usplit, (1, 3, 1))  # [B, 45, N]  (uh um ul) x3
    vr3 = np.concatenate([vh, vh, vh, vm, vm, vm, vl, vl, vl], axis=1)  # [B, 45, M]
    c = np.ascontiguousarray
    return [
        {
            "ut3": c(ut3[i * BPC : (i + 1) * BPC]),
            "vr3": c(vr3[i * BPC : (i + 1) * BPC]),
        }
        for i in range(NCORES)
    ]


def _run(in_maps, trace=False, repeat=1):
    from concourse.bass_utils import run_bass_kernel_spmd

    if repeat not in _built:
        _built[repeat] = _build_nc(repeat)
    return run_bass_kernel_spmd(
        _built[repeat], in_maps, list(range(NCORES)), trace=trace
    )


def _postprocess(results):
    out = np.empty((B,), np.float32)
    for c in range(NCORES):
        ra = results[c]["rowacc"]  # [BPC, 128, NCHUNK]
        ca = results[c]["colacc"]  # [BPC, 128, M//P]
        for b in range(BPC):
            out[c * BPC + b] = np.float32(
                ra[b].mean(dtype=np.float64) + ca[b].mean(dtype=np.float64)
            )
    return out


def kernel(x1: np.ndarray, x2: np.ndarray) -> np.ndarray:
    res = _run(_prep_in_maps(x1, x2))
    return _postprocess(res.results)
